# revision 29
# baseline (speedup 1.0000x reference)
"""Trainium2 Bass kernel: AnchorTargetLayer pairwise IoU overlaps.

Computes overlaps[b, n, g] for all (anchor, gt) pairs, with the reference's
zero-area / out-of-image masking, on 8 NeuronCores (anchor-axis sharded).

Math: with inter = iw*ih and S = a_area + g_area,
    IoU = inter / (S - inter) = 1/(1 - w) - 1     where w = inter/S.
iw factors over (shift_x=j, anchor=a, gt) and ih over (shift_y=i, anchor=a, gt),
so the device only computes, per output element:
    w = iw''(j,a,gt) * ih(i,a,gt)        (VectorE tensor_tensor mul)
    r = Reciprocal(-w + 1)               (ScalarE activation, free affine)
    out = r + negm[row]                  (VectorE tensor_scalar / ScalarE add)
iw'' = iw/S is host-precomputed (tiny: 576 x 1024), ih is replicated across
partitions by TensorE selector-matmuls into PSUM (bf16 hi+lo split, exact to
~4e-6 rel).  All masks are folded in by zeroing iw''/ih rows/cols (w=0 -> r=1)
plus the per-row constant negm in {-R1, -1-R1}, where R1 is the hardware ACT
table's Reciprocal(1.0) so that masked/empty cells come out exactly 0 / -1.
"""

import numpy as np
import ml_dtypes

import concourse.bass as bass
import concourse.mybir as mybir
import concourse.tile as tile
from concourse.vector_clock import ScopedClock
from concourse.bass_utils import run_bass_kernel_spmd

# Problem shape (hardcoded per spec).
B, G, H, W, A = 8, 128, 64, 64, 9
FEAT_STRIDE = 16
N_CORES = 8
I_PER_CORE = H // N_CORES          # 8 shift-rows per core
ROWS_PER_I = W * A                 # 576 anchors per shift-row
NPC = I_PER_CORE * ROWS_PER_I      # 4608 anchors per core
N_TOTAL = H * W * A                # 36864
F = B * G                          # 1024 free dim (batch x gt)
N_FULL_C = 4                       # c-blocks 0..3 are full 128-row tiles
N_TILES = I_PER_CORE * N_FULL_C + I_PER_CORE // 2   # 32 full + 4 paired-half

# Hardware ACT-table Reciprocal(1.0) (measured on TRN2; used to calibrate the
# additive constant so w=0 cells give exactly 0 / -1).
R1 = np.float32(0.9999880194664001)

# Which tiles do the final add on DVE (True) vs ACT (False) - load balance.
ADD_ON_DVE = tuple((t * 7) % 13 < 7 for t in range(N_TILES))

F32 = mybir.dt.float32
BF16 = mybir.dt.bfloat16
AFT = mybir.ActivationFunctionType

_CACHE = {}


def _patched_drain_and_barrier(self, tick_clock, wait_clock):
    """Tile tail-drain carrying N sem waits trips a walrus limit (>2 sync wait
    commands per ctrl instruction).  Spread the waits over one drain each."""
    nc = self.nc
    drain_inst = nc.sync.drain()
    wait_clock.add_sem_waits(
        drain_inst.ins, ScopedClock({None: tick_clock.global_clock})
    )
    si = drain_inst.ins.sync_info
    waits = list(si.on_wait) if si is not None and si.on_wait else []
    if len(waits) > 1:
        si.on_wait = waits[:1]
        for w in waits[1:]:
            d2 = nc.sync.drain()
            s2 = d2.ins.sync_info
            if s2 is None:
                d2.ins.sync_info = mybir.SyncInfo(on_wait=[w], on_update=[])
            else:
                s2.on_wait = [w]
    nc.all_engine_barrier()
    popped = nc._tile_sem_poison_stack.pop()
    assert popped is self._sem_poison
    nc.clear_and_free_semaphores(list(self.sems.allocated().values()))
    nc.all_engine_barrier()


def _install_patch():
    if not _CACHE.get("patched"):
        tile.TileContext._drain_and_barrier = _patched_drain_and_barrier
        _CACHE["patched"] = True


def _split_sync_waits(nc, max_waits=1):
    """This toolchain's walrus rejects instructions carrying more than
    `max_waits` sem waits.  Move excess waits onto same-engine NoOps inserted
    immediately before the instruction (in-order engines: semantics equal)."""
    n = 0
    for fn in nc.m.functions:
        for bb in fn.blocks:
            out = []
            for inst in bb.instructions:
                si = inst.sync_info
                if si is not None and si.on_wait and len(si.on_wait) > max_waits:
                    waits = list(si.on_wait)
                    si.on_wait = waits[:max_waits]
                    extra = waits[max_waits:]
                    for i in range(0, len(extra), max_waits):
                        nop = mybir.InstNoOp(
                            name=f"wsplit_{n}", engine=inst.engine, ins=[], outs=[],
                            sync_info=mybir.SyncInfo(
                                on_wait=extra[i:i + max_waits], on_update=[]
                            ),
                        )
                        n += 1
                        out.append(nop)
                out.append(inst)
            bb.instructions = out


def _raw_reciprocal(nc, out_ap, in_ap, scale=-1.0, bias=1.0):
    """ScalarE ACTIVATE(func=Reciprocal): out = 1/(scale*in + bias).
    The bass wrapper rejects Reciprocal (generic accuracy concerns); our input
    domain is [0.5, 1] -> r in [1, 2] where the table is ~1e-5 accurate."""
    eng = nc.scalar
    ins_l = [
        eng.lower_ap(in_ap),
        mybir.ImmediateValue(dtype=F32, value=float(bias)),
        mybir.ImmediateValue(dtype=F32, value=float(scale)),
        mybir.ImmediateValue(dtype=F32, value=0.0),
    ]
    return eng.add_instruction(
        mybir.InstActivation(
            name=nc.get_next_instruction_name(),
            func=AFT.Reciprocal,
            ins=ins_l,
            outs=[eng.lower_ap(out_ap)],
        )
    )


def _build_nc(timing=False, reps=1, loop_n=1, opts=None):
    """Build the SPMD per-core Bass graph (same graph on all 8 cores).

    timing=True: the big output goes to an internal DRAM tensor (same DMA
    work) and a tiny tok->tok_out passthrough is added; with reps=K the tile
    loop is emitted K times, and with loop_n=L a hardware For_i loop repeats
    it L times, so one NEFF execution carries K*L kernel bodies (wall-clock
    benchmarking without NTFF profiling).  opts: ablation/tuning knobs."""
    opts = dict(opts or {})
    no_out_dma = opts.pop("no_out_dma", False)
    dma_only = opts.pop("dma_only", False)
    work_bufs = opts.pop("work_bufs", 6)
    psum_bufs = opts.pop("psum_bufs", 3)
    adds = opts.pop("adds", "split")          # "split" | "dve" | "act"
    no_recip = opts.pop("no_recip", False)
    out_nbg = opts.pop("out_nbg", False)      # contiguous [N,B,G] out (host transposes)
    micro = opts.pop("micro", None)           # per-op microbench: tt|mm_tt|recip|add
    assert not opts, opts
    key = ("nc", timing, reps, loop_n, no_out_dma, dma_only, work_bufs, psum_bufs,
           adds, no_recip, out_nbg, micro)
    if key in _CACHE:
        return _CACHE[key]
    _install_patch()
    from contextlib import ExitStack

    nc = bass.Bass("TRN2", target_bir_lowering=False, debug=False)
    iw2_d = nc.declare_dram_parameter("iw2", [128, 5 * F], F32, isOutput=False)
    ihhi_d = nc.declare_dram_parameter("ihhi", [A, I_PER_CORE * F], BF16, isOutput=False)
    ihlo_d = nc.declare_dram_parameter("ihlo", [A, I_PER_CORE * F], BF16, isOutput=False)
    sel_d = nc.declare_dram_parameter("sel", [A, 6 * 128], BF16, isOutput=False)
    negm_d = nc.declare_dram_parameter("negm", [128, N_TILES], F32, isOutput=False)
    out_shape = [NPC, B, G] if out_nbg else [B, NPC, G]
    if timing:
        tok_d = nc.declare_dram_parameter("tok", [1, 4], F32, isOutput=False)
        toko_d = nc.declare_dram_parameter("tok_out", [1, 4], F32, isOutput=True)
        out_d = nc.dram_tensor("outbig", out_shape, F32)
    else:
        out_d = nc.declare_dram_parameter("out", out_shape, F32, isOutput=True)

    with tile.TileContext(nc) as tc, ExitStack() as ctx:
        consts = ctx.enter_context(tc.tile_pool(name="consts", bufs=1))
        psums = ctx.enter_context(tc.tile_pool(name="psums", bufs=psum_bufs, space="PSUM"))
        work = ctx.enter_context(tc.tile_pool(name="work", bufs=work_bufs))

        iw2_sb = consts.tile([128, 5 * F], F32)
        nc.sync.dma_start(out=iw2_sb[:], in_=iw2_d.ap())
        ihhi_sb = consts.tile([A, I_PER_CORE * F], BF16)
        nc.sync.dma_start(out=ihhi_sb[:], in_=ihhi_d.ap())
        ihlo_sb = consts.tile([A, I_PER_CORE * F], BF16)
        nc.sync.dma_start(out=ihlo_sb[:], in_=ihlo_d.ap())
        sel_sb = consts.tile([A, 6 * 128], BF16)
        nc.sync.dma_start(out=sel_sb[:], in_=sel_d.ap())
        negm_sb = consts.tile([128, N_TILES], F32)
        nc.sync.dma_start(out=negm_sb[:], in_=negm_d.ap())
        if timing:
            tok_sb = consts.tile([1, 4], F32)
            nc.sync.dma_start(out=tok_sb[:], in_=tok_d.ap())
            nc.sync.dma_start(out=toko_d.ap(), in_=tok_sb[:])

        out_ap = out_d.ap()

        def emit_micro(t_idx, c_block, mm_specs):
            cs = iw2_sb[:, c_block * F:(c_block + 1) * F]
            if micro == "tt":
                w_t = work.tile([128, F], F32, tag="w")
                nc.vector.tensor_mul(w_t[:], cs, iw2_sb[:, 0:F])
            elif micro == "mm_tt":
                ps = psums.tile([128, F], F32, tag="ihrep")
                for half in (0, 1):
                    fr = half * 512
                    n_mm = len(mm_specs) * 2
                    k = 0
                    for (cblk, il) in mm_specs:
                        lhsT = sel_sb[0:A, cblk * 128:(cblk + 1) * 128]
                        for src in (ihhi_sb, ihlo_sb):
                            rhs = src[0:A, il * F + fr: il * F + fr + 512]
                            nc.tensor.matmul(ps[:, fr:fr + 512], lhsT, rhs,
                                             start=(k == 0), stop=(k == n_mm - 1))
                            k += 1
                w_t = work.tile([128, F], F32, tag="w")
                nc.vector.tensor_mul(w_t[:], cs, ps[:])
            elif micro == "recip":
                r_t = work.tile([128, F], F32, tag="r")
                _raw_reciprocal(nc, r_t[:], cs)
            elif micro == "add":
                f_t = work.tile([128, F], F32, tag="f")
                nc.vector.tensor_scalar_add(f_t[:], cs, negm_sb[:, t_idx:t_idx + 1])
            else:
                raise ValueError(micro)

        def emit_tile(t_idx, c_block, mm_specs, out_dsts):
            if micro is not None:
                emit_micro(t_idx, c_block, mm_specs)
                return
            """mm_specs: list of (sel_colblock, i_local) matmul sources
            accumulated into one PSUM [128, F] replication of ih.
            out_dsts: list of (p0, p1, n0) output row ranges."""
            if dma_only:
                f_t = work.tile([128, F], F32, tag="f")
                nc.vector.memset(f_t[:], -1.0)
            else:
                ps = psums.tile([128, F], F32, tag="ihrep")
                for half in (0, 1):
                    fr = half * 512
                    n_mm = len(mm_specs) * 2
                    k = 0
                    for (cblk, il) in mm_specs:
                        lhsT = sel_sb[0:A, cblk * 128:(cblk + 1) * 128]
                        for src in (ihhi_sb, ihlo_sb):
                            rhs = src[0:A, il * F + fr: il * F + fr + 512]
                            nc.tensor.matmul(
                                ps[:, fr:fr + 512], lhsT, rhs,
                                start=(k == 0), stop=(k == n_mm - 1),
                            )
                            k += 1
                w_t = work.tile([128, F], F32, tag="w")
                nc.vector.tensor_mul(
                    w_t[:], iw2_sb[:, c_block * F:(c_block + 1) * F], ps[:]
                )
                if no_recip:
                    r_t = w_t
                else:
                    r_t = work.tile([128, F], F32, tag="r")
                    _raw_reciprocal(nc, r_t[:], w_t[:])
                f_t = work.tile([128, F], F32, tag="f")
                nb = negm_sb[:, t_idx:t_idx + 1]
                on_dve = {"split": ADD_ON_DVE[t_idx], "dve": True, "act": False}[adds]
                if on_dve:
                    nc.vector.tensor_scalar_add(f_t[:], r_t[:], nb)
                else:
                    nc.scalar.activation(f_t[:], r_t[:], AFT.Identity, bias=nb, scale=1.0)
            if not no_out_dma:
                for (p0, p1, n0) in out_dsts:
                    src = f_t[p0:p1, :].rearrange("p (b g) -> p b g", b=B)
                    if out_nbg:
                        dst = out_ap[n0:n0 + (p1 - p0), :, :]
                    else:
                        dst = out_ap[:, n0:n0 + (p1 - p0), :].rearrange("b p g -> p b g")
                    nc.sync.dma_start(out=dst, in_=src)

        def emit_body():
            for _rep in range(reps):
                t_idx = 0
                for il in range(I_PER_CORE):
                    for c in range(N_FULL_C):
                        n0 = il * ROWS_PER_I + c * 128
                        emit_tile(t_idx, c, [(c, il)], [(0, 128, n0)])
                        t_idx += 1
                for pi in range(I_PER_CORE // 2):
                    il0, il1 = 2 * pi, 2 * pi + 1
                    emit_tile(
                        t_idx, 4,
                        [(4, il0), (5, il1)],
                        [(0, 64, il0 * ROWS_PER_I + 512),
                         (64, 128, il1 * ROWS_PER_I + 512)],
                    )
                    t_idx += 1
                assert t_idx == N_TILES

        if loop_n > 1:
            with tc.For_i(0, loop_n, 1):
                emit_body()
        else:
            emit_body()

    _split_sync_waits(nc)
    _CACHE[key] = nc
    return nc


def _host_precompute(gt_boxes, im_info, anchors):
    """Mirror the reference's fp32 arithmetic; build per-core input maps."""
    f32 = np.float32
    anchors = np.asarray(anchors, dtype=f32)
    assert anchors.shape == (A, 4), anchors.shape
    gt = np.asarray(gt_boxes, dtype=f32)[:, :, :4]
    im_info = np.asarray(im_info, dtype=f32)
    im_h, im_w = im_info[0, 0], im_info[0, 1]

    sx = (np.arange(W, dtype=f32) * f32(FEAT_STRIDE))
    sy = (np.arange(H, dtype=f32) * f32(FEAT_STRIDE))
    # x-coords depend on (j, a) only; y-coords on (i, a) only (fp32-exact:
    # identical operands -> identical rounding for every i resp. j).
    ax1 = anchors[None, :, 0] + sx[:, None]     # [j, a]
    ax2 = anchors[None, :, 2] + sx[:, None]
    ay1 = anchors[None, :, 1] + sy[:, None]     # [i, a]
    ay2 = anchors[None, :, 3] + sy[:, None]
    aw = ax2 - ax1 + f32(1.0)                   # [j, a]
    ah = ay2 - ay1 + f32(1.0)                   # [i, a]
    x_ok = (ax1 >= f32(0.0)) & (ax2 < im_w)     # [j, a]
    y_ok = (ay1 >= f32(0.0)) & (ay2 < im_h)     # [i, a]

    gx1 = gt[..., 0].reshape(F)
    gy1 = gt[..., 1].reshape(F)
    gx2 = gt[..., 2].reshape(F)
    gy2 = gt[..., 3].reshape(F)
    gw = gx2 - gx1 + f32(1.0)
    gh = gy2 - gy1 + f32(1.0)
    g_area = gw * gh                            # [F]
    gt_zero = (gw == f32(1.0)) & (gh == f32(1.0))

    # a_area depends only on a when the coordinate sums are fp32-exact (true
    # for the reference generator); tiny deviations otherwise only perturb S.
    a_area9 = (aw[0] * ah[0]).astype(np.float64)          # [A]
    S = a_area9[:, None] + g_area[None, :].astype(np.float64)   # [A, F]

    # iw''[(j,a), F] = iw / S, with x-side + gt masks folded in.
    ax1f = ax1.reshape(ROWS_PER_I, 1)
    ax2f = ax2.reshape(ROWS_PER_I, 1)
    ix1 = np.maximum(ax1f, gx1[None, :])
    ix2 = np.minimum(ax2f, gx2[None, :])
    iw = np.maximum(ix2 - ix1 + f32(1.0), f32(0.0))       # [576, F] f32
    S_row = S[np.arange(ROWS_PER_I) % A]                  # [576, F]
    iw2 = (iw.astype(np.float64) / S_row).astype(f32)
    iw2[~x_ok.reshape(ROWS_PER_I), :] = 0.0
    iw2[:, gt_zero] = 0.0

    # ih[i, a, F] with y-side mask folded in.
    iy1 = np.maximum(ay1[:, :, None], gy1[None, None, :])
    iy2 = np.minimum(ay2[:, :, None], gy2[None, None, :])
    ih = np.maximum(iy2 - iy1 + f32(1.0), f32(0.0))       # [H, A, F] f32
    ih[~y_ok] = 0.0

    # Per-anchor additive constant (+ exact-zero calibration).
    inside = x_ok[None, :, :] & y_ok[:, None, :]          # [i, j, a]
    negm = np.where(inside, f32(-1.0) * R1, f32(-1.0) - R1).astype(f32)
    negm_n = negm.reshape(N_TOTAL)

    # Degenerate-anchor rows (aw==1 & ah==1) are fixed up on the host.
    a_zero = (aw[None, :, :] == f32(1.0)) & (ah[:, None, :] == f32(1.0))
    a_zero_n = a_zero.reshape(N_TOTAL)

    # ---- pack device tensors ----
    iw2_blocks = np.zeros((128, 5 * F), f32)
    for c in range(N_FULL_C):
        iw2_blocks[:, c * F:(c + 1) * F] = iw2[c * 128:(c + 1) * 128]
    iw2_blocks[0:64, 4 * F:] = iw2[512:576]
    iw2_blocks[64:128, 4 * F:] = iw2[512:576]

    sel = np.zeros((A, 6 * 128), f32)
    for c in range(N_FULL_C):
        p = np.arange(128)
        sel[(c * 128 + p) % A, c * 128 + p] = 1.0
    p = np.arange(64)
    sel[(512 + p) % A, 4 * 128 + p] = 1.0        # pair-lo: partitions 0..63
    sel[(512 + p) % A, 5 * 128 + 64 + p] = 1.0   # pair-hi: partitions 64..127
    sel_bf = sel.astype(ml_dtypes.bfloat16)

    in_maps = []
    for k in range(N_CORES):
        ih_core = np.ascontiguousarray(
            ih[k * I_PER_CORE:(k + 1) * I_PER_CORE].transpose(1, 0, 2)
        ).reshape(A, I_PER_CORE * F)                      # [A, il*F + f]
        ih_hi = ih_core.astype(ml_dtypes.bfloat16)
        ih_lo = (ih_core - ih_hi.astype(f32)).astype(ml_dtypes.bfloat16)

        nm = negm_n[k * NPC:(k + 1) * NPC]
        negm_sb = np.zeros((128, N_TILES), f32)
        t = 0
        for il in range(I_PER_CORE):
            for c in range(N_FULL_C):
                negm_sb[:, t] = nm[il * ROWS_PER_I + c * 128: il * ROWS_PER_I + (c + 1) * 128]
                t += 1
        for pi in range(I_PER_CORE // 2):
            negm_sb[0:64, t] = nm[(2 * pi) * ROWS_PER_I + 512:(2 * pi) * ROWS_PER_I + 576]
            negm_sb[64:128, t] = nm[(2 * pi + 1) * ROWS_PER_I + 512:(2 * pi + 1) * ROWS_PER_I + 576]
            t += 1

        in_maps.append({
            "iw2": iw2_blocks,
            "ihhi": ih_hi,
            "ihlo": ih_lo,
            "sel": sel_bf,
            "negm": negm_sb,
        })
    return in_maps, a_zero_n


# Contiguous [N,B,G] device layout measured SLOWER end-to-end (100.9us vs
# 74.1us) despite cheaper DMA in isolation (49.7 vs 67us) - some DMA/engine
# contention interaction.  Keep the [B,N,G]-scatter output APs.
OUT_NBG = False


def run_device(gt_boxes, im_info, anchors, trace=False, **kw):
    """Build inputs, run the SPMD kernel, gather the full output."""
    in_maps, a_zero_n = _host_precompute(gt_boxes, im_info, anchors)
    nc = _build_nc(opts={"out_nbg": OUT_NBG} if OUT_NBG else None)
    res = run_bass_kernel_spmd(
        nc, in_maps, core_ids=list(range(N_CORES)), trace=trace, **kw
    )
    out = np.empty((B, N_TOTAL, G), np.float32)
    for k in range(N_CORES):
        o = res.results[k]["out"]
        if OUT_NBG:
            o = o.transpose(1, 0, 2)
        out[:, k * NPC:(k + 1) * NPC, :] = o
    if a_zero_n.any():
        out[:, a_zero_n, :] = -1.0
    return out, res


def kernel(rpn_cls_score=None, gt_boxes=None, im_info=None, num_boxes=None,
           anchors=None):
    out, _ = run_device(gt_boxes, im_info, anchors)
    return out
